# revision 1
# baseline (speedup 1.0000x reference)
"""Location-dependent 3D conv (AsymConv) on 8 TRN2 NeuronCores.

Math (per output voxel):
    out[b, 0, x, y, z] = sum_{i,j,l in 0..2} Xp[b, x+i, y+j, z+l] * W[x, y, z, (i*3+j)*3+l]
with Xp = edge-padded X by 1 plane on each spatial side.

Strategy:
  - Shard the X spatial axis (96 = 8 cores x 12 planes). Host slices overlapping
    halo windows (14 planes) per core -> no inter-core communication at all.
  - Per core, SBUF layout: partition dim = y (96 used of 128), free = (b, x, z).
    Compute-engine APs must start at partition 0/32/64/96, so the y-shift cannot
    be a partition offset: the host ships 3 y-pre-shifted copies of the (small)
    X shard instead. The x/z shifts are plain free-dim AP offsets.
  - Products patch*W run on the Vector engine in fp16 (2x perf mode needs
    4-byte-aligned starts, so taps with l==1 read from a z-shifted copy made
    on the otherwise-idle ScalarE; those taps are issued last to hide the
    copies). The (l=0, l=2) taps of each (i, j) are fused into one DVE op via
    an overlapping stride-2 access-pattern dim, and the l=1 taps pair on the
    x-axis the same way (30 ops instead of 54 - per-op overhead and semaphore
    traffic were ~15% of Vector-engine time).
  - The 27-term accumulation runs on the otherwise-idle TensorEngine as
    identity-matmuls accumulating into PSUM (fp32), freeing the Vector engine
    from the adds.
  - W (the dominant 6 MB stream) moves as 27 per-tap DMAs, interleaved with
    the X planes in consumption order.
  - Interleaved schedule: the Vector engine computes both batches' products
    per tap (1.5 us of work per arriving W slot vs ~1 us arrival cadence, so
    it runs dense instead of stalling on the W stream), while the PE consumes
    products in groups of 4 taps per batch - switching PSUM bank groups only
    every 12 matmuls, since per-tap switching de-pipelines the PE.
  - PSUM -> SBUF fp16 (ScalarE) -> DRAM; host upcasts and reassembles.
"""

import os

import numpy as np

# ---- problem constants (hardcoded per harness rules) ----
B = 2
D = 96  # Dx = Dy = Dz
KSZ = 3
NTAP = KSZ**3  # 27
NCORES = 8
XS = D // NCORES  # 12 x-planes per core
XH = XS + 2  # with halo
ZP = D + 2  # padded z

F16 = np.float16
LAST_RESULT = None  # BassKernelResults of the most recent run (for test.py)

_GRAPH_CACHE = {}

N_WARMUP = int(os.environ.get("ASYM_WARMUP", "0"))
W_RING2 = bool(int(os.environ.get("ASYM_W_RING2", "0")))
# batch-1 taps computed on the otherwise-idle GPSIMD engine while batch 0's
# W-arrival-paced phase leaves DVE slack
N_GPS = int(os.environ.get("ASYM_GPS", "0"))

# taps with l != 1 are 4B-aligned in the base copies; issue them first so the
# ScalarE z-shift copies (needed by l == 1 taps) are off the critical path.
# Within each group, order by j so the y-shifted x1/x2 tiles (which land a few
# us after x0) are not needed until well into the tap stream.
TAP_ORDER = [
    t
    for lgroup in (False, True)
    for j in range(KSZ)
    for t in range(NTAP)
    if (t % 3 == 1) == lgroup and (t // 3) % 3 == j
]


def _build_graph():
    """Build (and cache) the per-core Bass graph. Same graph for all 8 cores."""
    if "nc" in _GRAPH_CACHE:
        return _GRAPH_CACHE["nc"]

    from concourse import bacc
    import concourse.mybir as mybir
    from concourse.tile import TileContext

    f16 = mybir.dt.float16
    f32 = mybir.dt.float32

    nc = bacc.Bacc("TRN2", target_bir_lowering=False, debug=False, num_devices=NCORES)

    # y-pre-shifted X copies: xj[y', b, x, z] = Xp[y'+j, b, x, z]
    x_ds = [
        nc.dram_tensor(f"x{j}", [D, B, XH, ZP], f16, kind="ExternalInput")
        for j in range(KSZ)
    ]
    w_d = nc.dram_tensor("w", [NTAP, D, XS, D], f16, kind="ExternalInput")
    id_d = nc.dram_tensor("ident", [D, D], f16, kind="ExternalInput")
    out_d = nc.dram_tensor("out", [D, B, XS, D], f16, kind="ExternalOutput")

    # x-chunks for PSUM banks: each chunk's fp32 free size must fit one 2KB bank
    CH = [(0, 5), (5, 5), (10, 2)]

    with TileContext(nc) as tc:
        with (
            tc.tile_pool(name="xp", bufs=1) as xpool,
            tc.tile_pool(name="wp", bufs=1) as wpool,
            tc.tile_pool(name="pp", bufs=4) as ppool,
            tc.tile_pool(name="psp", bufs=1, space="PSUM") as pspool,
        ):
            from concourse import bass as _bass

            # W tiles: taps TAP_ORDER[0..17] are (l=0, l=2) pairs sharing one
            # [D, 2, XS, D] tile (consumed by a single merged DVE op); the
            # l=1 taps TAP_ORDER[18..26] get single tiles
            w_pair = {}  # pair index p -> tile; pair p covers tn = 2p, 2p+1
            w_ipair = {}  # l=1 (i=0, i=1) pairs, key k = j
            w_single = {}  # tn -> tile

            def dma_w(wi):
                t = TAP_ORDER[wi]
                # first W transfers ride the ACT ring, landing in parallel
                # with the X halves on the SP ring -> earlier first product
                q = nc.scalar if (wi < 4 or (W_RING2 and wi % 2)) else nc.sync
                if wi < 2:
                    wt = wpool.tile([D, XS, D], f16, name=f"w_{t}", tag=f"w_{t}")
                    q.dma_start(out=wt[:], in_=w_d.ap()[t])
                    w_single[wi] = wt
                elif wi < 18:
                    p, s = (wi - 2) // 2, (wi - 2) % 2
                    if p not in w_pair:
                        w_pair[p] = wpool.tile(
                            [D, 2, XS, D], f16, name=f"wp_{p}", tag=f"wp_{p}"
                        )
                    q.dma_start(
                        out=w_pair[p][:, s : s + 1],
                        in_=w_d.ap()[t : t + 1].transpose([1, 0, 2, 3]),
                    )
                elif (wi - 18) % 3 < 2:
                    k, s_ = (wi - 18) // 3, (wi - 18) % 3
                    if k not in w_ipair:
                        w_ipair[k] = wpool.tile(
                            [D, 2, XS, D], f16, name=f"wq_{k}", tag=f"wq_{k}"
                        )
                    q.dma_start(
                        out=w_ipair[k][:, s_ : s_ + 1],
                        in_=w_d.ap()[t : t + 1].transpose([1, 0, 2, 3]),
                    )
                else:
                    wt = wpool.tile([D, XS, D], f16, name=f"w_{t}", tag=f"w_{t}")
                    q.dma_start(out=wt[:], in_=w_d.ap()[t])
                    w_single[wi] = wt

            def lpair_ap(j, b, i):
                """[D, 2, XS, D] view of x_ts[j]: overlapping z-windows l=0,2."""
                base = x_ts[j][:, b, i : i + XS, 0:D]
                ap = list(base.ap)
                return _bass.AP(
                    base.tensor, base.offset, [ap[0], [2, 2], ap[1], ap[2]]
                )

            def ipair_ap(j, b):
                """[D, 2, XS, D] view of x1_ts[j]: overlapping x-windows i=0,1."""
                base = x1_ts[j][:, b, 0:XS, 0:D]
                ap = list(base.ap)
                return _bass.AP(
                    base.tensor, base.offset, [ap[0], [ap[1][0], 2], ap[1], ap[2]]
                )

            # DMA order: x planes interleaved with the first W taps so the
            # first products can start early; the W flood follows and keeps
            # all 16 SDMA engines saturated
            x_ts = []  # base copies, z offset parity 0 (l = 0, 2)
            for j in range(KSZ):
                xt = xpool.tile([D, B, XH, ZP], f16, name=f"x_{j}", tag=f"x_{j}")
                nc.sync.dma_start(out=xt[:, 0:1], in_=x_ds[j].ap()[:, 0:1])
                dma_w(2 * j)
                dma_w(2 * j + 1)
                nc.sync.dma_start(out=xt[:, 1:2], in_=x_ds[j].ap()[:, 1:2])
                x_ts.append(xt)
            id_t = xpool.tile([D, D], f16, name="id_t", tag="id_t")
            nc.sync.dma_start(out=id_t[:], in_=id_d.ap())
            for wi in range(6, NTAP):
                dma_w(wi)

            x1_ts = []  # z-shifted by 1 (l = 1)
            for j in range(KSZ):
                x1 = xpool.tile([D, B, XH, ZP - 1], f16, name=f"xz_{j}", tag=f"xz_{j}")
                nc.scalar.copy(out=x1[:], in_=x_ts[j][:, :, :, 1:ZP])
                x1_ts.append(x1)

            if N_WARMUP:
                dummy = ppool.tile([D, 480], f16, name="dummy", tag="warm_rhs", bufs=1)
                nc.vector.memset(dummy[:], 0.0)
                ps_w = pspool.tile([D, 480], f32, name="ps_warm", tag="ps_warm")
                for _ in range(N_WARMUP):
                    nc.tensor.matmul(ps_w[:], id_t[:], dummy[:], start=True, stop=True)

            # Interleaved schedule: DVE alternates b0/b1 products per tap, so
            # each arriving W slot unlocks 1.5us of DVE work vs ~1us arrival
            # cadence (DVE runs dense instead of stalling on the W stream).
            # PE consumes in groups of GROUP taps per batch, switching PSUM
            # bank groups only every 3*GROUP matmuls -- frequent per-tap
            # switches measurably de-pipeline the PE.
            GROUP = 4
            psums = {
                (b, ci): pspool.tile(
                    [D, nx, D], f32, name=f"ps_{b}_{ci}", tag=f"ps_{b}_{ci}"
                )
                for b in range(B)
                for ci, (x0, nx) in enumerate(CH)
            }
            # units: 9 (l=0,l=2) pairs covering tn 0..17, then 9 singles
            units = [("s", (0,)), ("s", (1,))]
            units += [("p", (2 + 2 * p, 3 + 2 * p)) for p in range(8)]
            for k in range(3):
                units.append(("q", (18 + 3 * k, 18 + 3 * k + 1)))
                units.append(("s", (18 + 3 * k + 2,)))
            UG = 2  # units per PE flush group (~12 matmuls per bank switch)
            for g0 in range(0, len(units), UG):
                gunits = units[g0 : g0 + UG]
                prods = {}
                for ui, (kind, tns) in enumerate(gunits, start=g0):
                    t = TAP_ORDER[tns[0]]
                    i, j, l = t // 9, (t // 3) % 3, t % 3
                    for b in range(B):
                        if kind == "p":
                            prod = ppool.tile(
                                [D, 2, XS, D], f16, name="prodp", tag="prodp", bufs=5
                            )
                            nc.vector.tensor_mul(
                                out=prod[:],
                                in0=lpair_ap(j, b, i),
                                in1=w_pair[(tns[0] - 2) // 2][:],
                            )
                        elif kind == "q":
                            prod = ppool.tile(
                                [D, 2, XS, D], f16, name="prodp", tag="prodp", bufs=5
                            )
                            nc.vector.tensor_mul(
                                out=prod[:],
                                in0=ipair_ap(j, b),
                                in1=w_ipair[(tns[0] - 18) // 3][:],
                            )
                        else:
                            src = (
                                x_ts[j][:, b, i : i + XS, l : l + D]
                                if l != 1
                                else x1_ts[j][:, b, i : i + XS, 0:D]
                            )
                            prod = ppool.tile(
                                [D, XS, D], f16, name="prod", tag="prod", bufs=5
                            )
                            nc.vector.tensor_mul(
                                out=prod[:], in0=src, in1=w_single[tns[0]][:]
                            )
                        prods[(b, ui)] = prod
                for b in range(B):
                    for ui, (kind, tns) in enumerate(gunits, start=g0):
                        for si, tn in enumerate(tns):
                            rhs_src = (
                                prods[(b, ui)][:, si]
                                if kind in ("p", "q")
                                else prods[(b, ui)]
                            )
                            for ci, (x0, nx) in enumerate(CH):
                                nc.tensor.matmul(
                                    psums[(b, ci)][:],
                                    id_t[:],
                                    rhs_src[:, x0 : x0 + nx, :],
                                    start=(tn == 0),
                                    stop=(tn == NTAP - 1),
                                )
            for b in range(B):
                for ci, (x0, nx) in enumerate(CH):
                    outsb = ppool.tile(
                        [D, nx, D], f16, name="outsb", tag=f"outsb_{b}_{ci}", bufs=1
                    )
                    if ci % 2:
                        nc.vector.tensor_copy(out=outsb[:], in_=psums[(b, ci)][:])
                    else:
                        nc.scalar.copy(out=outsb[:], in_=psums[(b, ci)][:])
                    nc.sync.dma_start(
                        out=out_d.ap()[:, b, x0 : x0 + nx, :],
                        in_=outsb[:],
                    )

    nc.compile()
    _GRAPH_CACHE["nc"] = nc
    return nc


def make_in_maps(X, W):
    """Host-side shard prep. X [2,1,96,96,96] f32, W [1,1,96,96,96,27] f32."""
    X = np.asarray(X)
    W = np.asarray(W)
    Xs = X.reshape(B, D, D, D)
    # edge padding on all three spatial dims
    Xp = np.pad(Xs, ((0, 0), (1, 1), (1, 1), (1, 1)), mode="edge")
    # -> [y, b, x, z]
    Xt = np.ascontiguousarray(np.transpose(Xp, (2, 0, 1, 3))).astype(F16)
    W00 = W.reshape(D, D, D, NTAP)
    ident = np.eye(D, dtype=F16)

    in_maps = []
    for m in range(NCORES):
        xs_full = Xt[:, :, m * XS : m * XS + XH, :]  # [98, 2, 14, 98]
        im = {"ident": ident}
        for j in range(KSZ):
            im[f"x{j}"] = np.ascontiguousarray(xs_full[j : j + D])
        wm = W00[m * XS : (m + 1) * XS]  # [12, 96, 96, 27]
        im["w"] = np.ascontiguousarray(np.transpose(wm, (3, 1, 0, 2))).astype(F16)
        in_maps.append(im)
    return in_maps


def kernel(X, W):
    global LAST_RESULT
    from concourse.bass_utils import run_bass_kernel_spmd

    nc = _build_graph()
    in_maps = make_in_maps(X, W)
    trace = bool(int(os.environ.get("ASYM_TRACE", "0")))
    res = run_bass_kernel_spmd(
        nc, in_maps, core_ids=list(range(NCORES)), trace=trace
    )
    LAST_RESULT = res

    out = np.empty((B, 1, D, D, D), dtype=np.float32)
    for m in range(NCORES):
        r = res.results[m]["out"].astype(np.float32)  # [y, b, x, z]
        out[:, 0, m * XS : (m + 1) * XS, :, :] = np.transpose(r, (1, 2, 0, 3))
    return out

